# revision 1
# baseline (speedup 1.0000x reference)
"""Mixtral MoE MLP (T=8192, H=2048, I=4096, E=8, top-2) on 8 TRN2 NeuronCores.

Strategy: tensor-parallel over intermediate_size + selective fp8. Every core
holds a 512-wide I-shard of ALL 8 experts (SBUF resident per expert) and
processes ALL routed token-expert pairs, so per-core work is identical by
construction regardless of routing balance. Router + gathers + the final
top-2 weighted combine and cross-shard reduction run on host.

Token-expert pairs whose renormalized combine weight is < FP8_TH run
entirely in fp8-e4m3 with DoubleRow matmuls (2 MACs/PE/cycle): their ~5.9%
expert-output error is scaled by the small combine weight, adding ~1.3e-2
to the final relative error (gate 2e-2). All other pairs run in fp16.

x and y use STRIPE-BLOCKED DRAM layouts ([P, cols], each stripe's block
contiguous) so every stripe DMA is a single multi-KB contiguous run per
partition; SBUF tiles stay feature-major so matmul moving operands and PSUM
drains are contiguous. GEMM2 of each stripe is issued after GEMM1 of the
next stripe so the PE never waits on the act drain (ACT+DVE) latency.

fp8 scales: x*16, w*512, act*4 (TRN e4m3 max normal is +-240, overflow ->
Inf, so everything is clipped host-side and the act scale keeps headroom);
the PSUM scale SA*SW is folded into the host combine weights.
"""

import numpy as np

T, H, I, E = 8192, 2048, 4096, 8
TOP_K = 2
P = 128
KH = H // P            # 16  K-tiles for GEMM1 (contraction over H)
KH2 = KH // 2          # 8   fp8 DoubleRow K-tile pairs
ISH = I // E           # 512 I-shard per core
NP4 = ISH // P         # 4   gate/up 128-row pair blocks per shard
KI4 = ISH // P         # 4   K-tiles for GEMM2 (contraction over I-shard)
KI8 = KI4 // 2         # 2   fp8 DoubleRow K-tile pairs for GEMM2
NH = H // P            # 16  output row blocks of GEMM2
BLOCK = 512            # max moving-operand / PSUM bank width

FP8_TH = 0.45          # combine-weight threshold for the fp8 path
FP8_CAP = 512          # per-expert fp8 token cap (single <=512-wide stripe)
SX = 16.0              # fp8 scale on x
SW = 512.0             # fp8 scale on ws/w2s
SA = 4.0               # fp8 scale on act
E4MAX = 240.0          # TRN e4m3 max normal

_module_cache = {}


def _stripes(C, first_small=False, last_small=False, align=1):
    """Split [0, C) into near-uniform aligned blocks of <= BLOCK tokens."""
    if C == 0:
        return []
    out = []
    off = 0
    tail = 0
    if first_small and C > 256:
        out.append((0, 128))
        off = 128
        C -= 128
    if last_small and C > 256:
        tail = 128
        C -= 128
    n_blocks = max(1, -(-C // BLOCK))
    base = C // (n_blocks * align) * align
    widths = [base] * n_blocks
    widths[-1] += C - base * n_blocks
    if tail:
        widths.append(tail)
    for w in widths:
        out.append((off, w))
        off += w
    return out


def _plan(Cs16, Cs8):
    """[(kind, expert, seg_off, xblk_off, yblk_off, width), ...] and totals.

    kind: 0 fp16, 1 fp8.  seg_off: token offset within the segment.
    xblk_off: element-column offset into the segment's x dram tensor.
    yblk_off: column offset into yt (NH-wide blocks).
    """
    plan = []
    x16c = x8c = yc = tok = 0
    seg_tok = {}     # (kind, e) -> global token offset of segment start
    for e in range(E):
        p16, p8 = [], []
        seg_tok[(0, e)] = tok
        for s_off, s_w in _stripes(Cs16[e], first_small=(e == 0),
                                   last_small=(e == E - 1)):
            p16.append((0, e, s_off, x16c, yc, s_w))
            x16c += KH * s_w
            yc += NH * s_w
        tok += Cs16[e]
        seg_tok[(1, e)] = tok
        for s_off, s_w in _stripes(Cs8[e], align=16):
            p8.append((1, e, s_off, x8c, yc, s_w))
            x8c += KH * s_w          # KH2*2 = KH cols per token (1B each)
            yc += NH * s_w
        tok += Cs8[e]
        # last expert: fp8 second-to-last so the kernel tail is the narrow
        # fp16 stripe (fp8 not first: its weights load behind e6's fp8)
        plan.extend(p16[:-1] + p8 + p16[-1:] if e == E - 1 else p16 + p8)
    return plan, x16c, x8c, yc, seg_tok


def _build_module(Cs16, Cs8):
    import concourse.mybir as mybir
    import concourse.tile as tile
    from concourse import bacc
    from contextlib import ExitStack

    fp16 = mybir.dt.float16
    fp32 = mybir.dt.float32
    fp8 = mybir.dt.float8e4
    DR = mybir.MatmulPerfMode.DoubleRow

    plan, x16cols, x8cols, ycols, _ = _plan(Cs16, Cs8)

    nc = bacc.Bacc("TRN2", target_bir_lowering=False, debug=False)

    xt = nc.dram_tensor("xt", [P, x16cols], fp16, kind="ExternalInput")
    w1 = nc.dram_tensor("w1", [E, P, NP4, KH, 2 * P], fp16, kind="ExternalInput")
    w2 = nc.dram_tensor("w2", [E, P, NH, KI4, P], fp16, kind="ExternalInput")
    if x8cols:
        xt8 = nc.dram_tensor("xt8", [P, x8cols], fp8, kind="ExternalInput")
        w18 = nc.dram_tensor("w18", [E, P, NP4, KH2, 2, 2 * P], fp8,
                             kind="ExternalInput")
        w28 = nc.dram_tensor("w28", [E, P, NH, KI8, 2, P], fp8,
                             kind="ExternalInput")
    yt = nc.dram_tensor("yt", [P, ycols], fp16, kind="ExternalOutput")

    act_fn = mybir.ActivationFunctionType.Silu
    copy_fn = mybir.ActivationFunctionType.Copy

    with tile.TileContext(nc) as tc, ExitStack() as ctx:
        xpool = ctx.enter_context(tc.tile_pool(name="xs", bufs=2))
        x8pool = ctx.enter_context(tc.tile_pool(name="x8s", bufs=2))
        apool = ctx.enter_context(tc.tile_pool(name="act", bufs=2))
        w1pool = ctx.enter_context(tc.tile_pool(name="w1p", bufs=2))
        w2pool = ctx.enter_context(tc.tile_pool(name="w2p", bufs=2))
        w18pool = ctx.enter_context(tc.tile_pool(name="w18p", bufs=1))
        w28pool = ctx.enter_context(tc.tile_pool(name="w28p", bufs=1))
        tpool = ctx.enter_context(tc.tile_pool(name="tmp", bufs=2))
        ypool = ctx.enter_context(tc.tile_pool(name="yst", bufs=2))
        ps1 = ctx.enter_context(tc.tile_pool(name="ps1", bufs=2, space="PSUM"))
        ps2 = ctx.enter_context(tc.tile_pool(name="ps2", bufs=3, space="PSUM"))

        pending = None   # (is8, w2tile, actT, yblk_off, width)

        def do_gemm2(is8, w2t, actT, y_off, g_w):
            for half in range(2):
                ys = ypool.tile([P, NH // 2, g_w], fp16)
                for hh in range(NH // 2):
                    h = half * (NH // 2) + hh
                    ps = ps2.tile([P, g_w], fp32)
                    if is8:
                        for k2 in range(KI8):
                            nc.tensor.matmul(
                                ps[:], w2t[:, h, k2, :, :], actT[:, k2, :, :],
                                start=(k2 == 0), stop=(k2 == KI8 - 1),
                                perf_mode=DR)
                    else:
                        for k2 in range(KI4):
                            nc.tensor.matmul(
                                ps[:], w2t[:, h, k2, :], actT[:, k2, :],
                                start=(k2 == 0), stop=(k2 == KI4 - 1))
                    nc.vector.tensor_copy(ys[:, hh, :], ps[:])
                c0 = y_off + half * (NH // 2) * g_w
                nc.scalar.dma_start(yt[:, c0:c0 + (NH // 2) * g_w], ys[:])

        cur_e = -1
        w1t = w2t = w18t = w28t = None
        for kind, e, s_off, x_off, y_off, s_w in plan:
            if e != cur_e:
                cur_e = e
                w1t = w1pool.tile([P, NP4, KH, 2 * P], fp16)
                w2t = w2pool.tile([P, NH, KI4, P], fp16)
                if e == 0:
                    # k-granular first pair so the PE can start asap
                    for k in range(KH):
                        nc.gpsimd.dma_start(w1t[:, 0, k, :], w1[e, :, 0, k, :])
                    for pr in range(1, NP4):
                        nc.gpsimd.dma_start(w1t[:, pr, :, :], w1[e, :, pr, :, :])
                else:
                    nc.gpsimd.dma_start(w1t[:], w1[e])
                nc.gpsimd.dma_start(w2t[:], w2[e])
                if Cs8[e]:
                    w18t = w18pool.tile([P, NP4, KH2, 2, 2 * P], fp8)
                    w28t = w28pool.tile([P, NH, KI8, 2, P], fp8)
                    nc.gpsimd.dma_start(w18t[:], w18[e])
                    nc.gpsimd.dma_start(w28t[:], w28[e])

            if kind == 0:
                xs = xpool.tile([P, KH, s_w], fp16)
                nc.sync.dma_start(xs[:], xt[:, x_off:x_off + KH * s_w])

                actT = apool.tile([P, KI4, s_w], fp16)
                for pr in range(NP4):
                    pg = ps1.tile([P, s_w], fp32)
                    pu = ps1.tile([P, s_w], fp32)
                    for k in range(KH):
                        nc.tensor.matmul(
                            pg[:], w1t[:, pr, k, 0:P], xs[:, k, :],
                            start=(k == 0), stop=(k == KH - 1))
                    for k in range(KH):
                        nc.tensor.matmul(
                            pu[:], w1t[:, pr, k, P:2 * P], xs[:, k, :],
                            start=(k == 0), stop=(k == KH - 1))
                    tmp = tpool.tile([P, s_w], fp32)
                    nc.scalar.activation(tmp[:], pg[:], act_fn)
                    nc.vector.tensor_mul(actT[:, pr, :], tmp[:], pu[:])
                nxt = (False, w2t, actT, y_off, s_w)
            else:
                xs8 = x8pool.tile([P, KH2, 2, s_w], fp8)
                nc.sync.dma_start(xs8[:], xt8[:, x_off:x_off + KH * s_w])

                a8 = apool.tile([P, KI8, 2, s_w], fp8)
                for pr in range(NP4):
                    pg = ps1.tile([P, s_w], fp32)
                    pu = ps1.tile([P, s_w], fp32)
                    for k in range(KH2):
                        nc.tensor.matmul(
                            pg[:], w18t[:, pr, k, :, 0:P], xs8[:, k, :, :],
                            start=(k == 0), stop=(k == KH2 - 1), perf_mode=DR)
                    for k in range(KH2):
                        nc.tensor.matmul(
                            pu[:], w18t[:, pr, k, :, P:2 * P], xs8[:, k, :, :],
                            start=(k == 0), stop=(k == KH2 - 1), perf_mode=DR)
                    # psum_g = SX*SW*gate; tmp = silu(gate); a8 = SA*act
                    tmp = tpool.tile([P, s_w], fp32)
                    nc.scalar.activation(tmp[:], pg[:], act_fn,
                                         scale=1.0 / (SX * SW))
                    tmp2 = tpool.tile([P, s_w], fp32)
                    nc.vector.tensor_mul(tmp2[:], tmp[:], pu[:])
                    nc.scalar.activation(a8[:, pr // 2, pr % 2, :], tmp2[:],
                                         copy_fn, scale=SA / (SX * SW))
                nxt = (True, w28t, a8, y_off, s_w)

            if pending is not None:
                do_gemm2(*pending)
            pending = nxt

        do_gemm2(*pending)

    nc.compile()
    return nc


def _route(hidden_states, router_w):
    """Replicate reference routing: softmax -> top-2 -> renormalize."""
    logits = hidden_states.astype(np.float64) @ router_w.astype(np.float64).T
    order = np.argsort(-logits, axis=1, kind="stable")
    top2 = order[:, :TOP_K]                                   # [T, 2]
    m = logits.max(axis=1, keepdims=True)
    p = np.exp(logits - m)
    p /= p.sum(axis=1, keepdims=True)
    w = np.take_along_axis(p, top2, axis=1)
    w = w / w.sum(axis=1, keepdims=True)                      # [T, 2]
    return top2, w


def _q8(a, scale):
    import ml_dtypes
    return np.clip(a * scale, -E4MAX, E4MAX).astype(ml_dtypes.float8_e4m3)


def _prep_w1(ws, core):
    # ws: [E, 2I, H] fp32 -> [E, P(part=H%128), NP4, KH, 256] fp16 for shard
    out = np.empty((E, P, NP4, KH, 2 * P), dtype=np.float16)
    lo, hi = core * ISH, (core + 1) * ISH
    for e in range(E):
        g = ws[e, lo:hi, :].astype(np.float16)          # [512, 2048]
        u = ws[e, I + lo:I + hi, :].astype(np.float16)
        # [pr, m, k, kp] -> [kp, pr, k, m]
        out[e, :, :, :, :P] = g.reshape(NP4, P, KH, P).transpose(3, 0, 2, 1)
        out[e, :, :, :, P:] = u.reshape(NP4, P, KH, P).transpose(3, 0, 2, 1)
    return out


def _prep_w2(w2s, core):
    # w2s: [E, H, I] fp32 -> [E, P(part=Ishard%128), NH, KI4, P(col=H%128)]
    out = np.empty((E, P, NH, KI4, P), dtype=np.float16)
    lo, hi = core * ISH, (core + 1) * ISH
    for e in range(E):
        s = w2s[e, :, lo:hi].astype(np.float16)         # [2048, 512]
        # [h, m, k2, kp] -> [kp, h, k2, m]
        out[e] = s.reshape(NH, P, KI4, P).transpose(3, 0, 2, 1)
    return out


def _prep_w18(ws, core):
    import ml_dtypes
    # -> [E, P, NP4, KH2, 2, 256] e4m3 (x SW)
    out = np.empty((E, P, NP4, KH2, 2, 2 * P), dtype=ml_dtypes.float8_e4m3)
    lo, hi = core * ISH, (core + 1) * ISH
    for e in range(E):
        g = _q8(ws[e, lo:hi, :], SW)                    # [512, 2048]
        u = _q8(ws[e, I + lo:I + hi, :], SW)
        # [pr, m, k8, pl, kp] -> [kp, pr, k8, pl, m]
        out[e, :, :, :, :, :P] = g.reshape(
            NP4, P, KH2, 2, P).transpose(4, 0, 2, 3, 1)
        out[e, :, :, :, :, P:] = u.reshape(
            NP4, P, KH2, 2, P).transpose(4, 0, 2, 3, 1)
    return out


def _prep_w28(w2s, core):
    import ml_dtypes
    # -> [E, P, NH, KI8, 2, P] e4m3 (x SW)
    out = np.empty((E, P, NH, KI8, 2, P), dtype=ml_dtypes.float8_e4m3)
    lo, hi = core * ISH, (core + 1) * ISH
    for e in range(E):
        s = _q8(w2s[e, :, lo:hi], SW)                   # [2048, 512]
        # [h, m, k8, pl, kp] -> [kp, h, k8, pl, m]
        out[e] = s.reshape(NH, P, KI8, 2, P).transpose(4, 0, 2, 3, 1)
    return out


def _ensure_ntff_hook():
    """Register the axon NTFF profile hook if the image's antenv lacks it."""
    import sys, types
    try:
        from antenv.axon_hooks import get_axon_ntff_profile_hook  # noqa: F401
        return
    except ImportError:
        pass
    try:
        from trn_agent_boot.trn_boot import _ntff_profile_via_ctypes
        hook = _ntff_profile_via_ctypes("/opt/axon/libaxon_pjrt.so")
    except Exception:
        hook = None
    mod = types.ModuleType("antenv.axon_hooks")
    mod.get_axon_ntff_profile_hook = lambda: hook
    mod.set_axon_ntff_profile_hook = lambda h: None
    sys.modules["antenv.axon_hooks"] = mod


def _pad(n, a):
    return max(a, -(-n // a) * a) if n else 0


def _run(hidden_states, router_w, ws, w2s, trace=False):
    from concourse.bass_utils import run_bass_kernel_spmd

    if trace:
        _ensure_ntff_hook()

    hidden_states = np.asarray(hidden_states, dtype=np.float32)
    router_w = np.asarray(router_w, dtype=np.float32)
    ws = np.asarray(ws, dtype=np.float32)
    w2s = np.asarray(w2s, dtype=np.float32)

    top2, topw = _route(hidden_states, router_w)

    idx16, wt16, idx8, wt8 = [], [], [], []
    for e in range(E):
        rows, which = np.nonzero(top2 == e)
        w = topw[rows, which]
        is8 = w < FP8_TH
        if is8.sum() > FP8_CAP:
            # keep only the FP8_CAP smallest weights in the fp8 tier so the
            # fp8 segment stays a single <=512-wide (LDW-bound) stripe
            cand = np.nonzero(is8)[0]
            drop = cand[np.argsort(w[cand])[FP8_CAP:]]
            is8[drop] = False
        idx16.append(rows[~is8])
        wt16.append(w[~is8])
        idx8.append(rows[is8])
        wt8.append(w[is8])

    Cs16 = tuple(_pad(len(ix), 8) for ix in idx16)
    Cs8 = tuple(_pad(len(ix), 16) for ix in idx8)

    key = (Cs16, Cs8)
    if key not in _module_cache:
        _module_cache[key] = _build_module(Cs16, Cs8)
    nc = _module_cache[key]

    plan, x16cols, x8cols, ycols, seg_tok = _plan(Cs16, Cs8)

    # token-segment bookkeeping (global token index per stripe for x packing
    # and per-segment slices for y decode)
    hidden16 = hidden_states.astype(np.float16)
    x16_rows = {}    # e -> [C16_e, H] fp16
    for e in range(E):
        seg = np.zeros((Cs16[e], H), dtype=np.float16)
        seg[:len(idx16[e])] = hidden16[idx16[e]]
        x16_rows[e] = seg
    xt = np.empty((P, x16cols), dtype=np.float16)
    for kind, e, s_off, x_off, y_off, s_w in plan:
        if kind != 0:
            continue
        blk = x16_rows[e][s_off:s_off + s_w]            # [n, H]
        xt[:, x_off:x_off + KH * s_w] = (
            blk.reshape(s_w, KH, P).transpose(2, 1, 0).reshape(P, KH * s_w))

    in_maps = [{
        "xt": xt,
        "w1": _prep_w1(ws, c),
        "w2": _prep_w2(w2s, c),
    } for c in range(E)]

    if x8cols:
        import ml_dtypes
        hidden8 = _q8(hidden_states, SX)
        x8_rows = {}
        for e in range(E):
            seg = np.zeros((Cs8[e], H), dtype=ml_dtypes.float8_e4m3)
            seg[:len(idx8[e])] = hidden8[idx8[e]]
            x8_rows[e] = seg
        xt8 = np.empty((P, x8cols), dtype=ml_dtypes.float8_e4m3)
        for kind, e, s_off, x_off, y_off, s_w in plan:
            if kind != 1:
                continue
            blk = x8_rows[e][s_off:s_off + s_w]         # [n, H] e4m3
            # [n, k8, pl, p] -> [p, k8, pl, n]
            xt8[:, x_off:x_off + KH * s_w] = (
                blk.reshape(s_w, KH2, 2, P).transpose(3, 1, 2, 0)
                .reshape(P, KH * s_w))
        w18s = [_prep_w18(ws, c) for c in range(E)]
        w28s = [_prep_w28(w2s, c) for c in range(E)]
        for c in range(E):
            in_maps[c]["xt8"] = xt8
            in_maps[c]["w18"] = w18s[c]
            in_maps[c]["w28"] = w28s[c]

    res = run_bass_kernel_spmd(nc, in_maps, core_ids=list(range(E)),
                               trace=trace)

    # host: reduce partial sums over I-shards, decode stripes, combine
    y_cols = np.zeros((P, ycols), dtype=np.float32)
    for c in range(E):
        y_cols += res.results[c]["yt"]

    Ctot = sum(Cs16) + sum(Cs8)
    y = np.empty((Ctot, H), dtype=np.float32)
    for kind, e, s_off, x_off, y_off, s_w in plan:
        blk = y_cols[:, y_off:y_off + NH * s_w].reshape(P, NH, s_w)
        t0 = seg_tok[(kind, e)] + s_off
        y[t0:t0 + s_w] = blk.transpose(2, 1, 0).reshape(s_w, H)

    out = np.zeros(hidden_states.shape, dtype=np.float32)
    inv8 = 1.0 / (SA * SW)          # fp8 segment psum carries SA*SW scale
    for e in range(E):
        if len(idx16[e]):
            t0 = seg_tok[(0, e)]
            seg = y[t0:t0 + len(idx16[e])]
            out[idx16[e]] += wt16[e][:, None].astype(np.float32) * seg
        if len(idx8[e]):
            t0 = seg_tok[(1, e)]
            seg = y[t0:t0 + len(idx8[e])]
            out[idx8[e]] += (wt8[e][:, None] * inv8).astype(np.float32) * seg
    return out, res


def kernel(hidden_states, router_w, ws, w2s):
    out, _ = _run(hidden_states, router_w, ws, w2s, trace=False)
    return out

